# revision 2
# baseline (speedup 1.0000x reference)
"""Trainium2 Bass kernel for nn_Dot_Attention — PE (TensorEngine) version.

The baseline computed alpha[m,s] = q[m]·V[m,s,:] on DVE via a custom
fused op at ~473 ns per [128,384] instruction (no 2x perf mode for
custom DVE ops) -> 424 us/core DVE-bound; DMA (219 us fp32) fully
hidden. This version moves the dot product to the idle TensorEngine:

  Host ships VT[m] = V[m].T as [ML, D, S] fp16 plus qT = q.T [D, ML].
  For each mention, PE computes alpha_T[s, m] = VT_chunk.T @ q_col by
  3 accumulating matmuls (K=128 d-chunk contraction, stationary =
  VT chunk [128 d, 128 s] fp16 with automatic Fast-Weight-Load at 2
  cols/cycle, moving = q column [128, 1]).  LDWEIGHTS bandwidth sets
  the floor: ~0.5 cyc/elem at 1.2 GHz => ~143 us/core for the trimmed
  stream, ~3x faster than the DVE path.

  Mentions are sorted by length and dealt round-robin so every core
  shares one module; per 128-slot the s-extent is rounded to 128
  (2..4 chunks). PSUM collects 32 mentions per group ([128 s, <=128
  cols]); a single DVE scalar_tensor_tensor per (group, chunk)
  evacuates PSUM -> SBUF fused with the alpha clamp (min 30) and the
  host-precomputed additive mask (-300 on invalid (m,s)).  Epilogue in
  the transposed layout: ACT exp, PE ones-matmul column sums, DVE
  reciprocal, PE rank-1 broadcast of 1/sum, DVE multiply.  Output
  leaves as out_T [S, ML] fp32; the host transposes it back.

VT is shipped packed 4-mentions-per-row-group ([ML/4, D, 4, S] fp16) so
each vt DMA moves 512 KB with 4 KB contiguous per partition — the
0.5-1 KB lines of the per-mention layout halved DMA throughput and made
the kernel DMA-bound (224 us); with 4 KB lines the ~100 MB/core stream
(~126 us at the measured ~800 GB/s/core) hides fully under the PE
weight-load stream.  Matmul accumulation is fp32; end-to-end rel err
~1.5e-3 (gate 2e-2).  Measured HW exec: ~138 us/core vs 439 us for the
DVE-custom-op baseline (PE-only probe: the 2688 matmul issues cost ~5 us
when weights are cached — LDWEIGHTS streaming is the true floor).
"""

import math

import numpy as np

M, S, D = 2048, 512, 384
NCORES = 8
ML = M // NCORES          # mentions per core
GRP = 32                  # mentions per PSUM group
NGRP = ML // GRP
DCH = D // 128            # d-chunks (3)
SCH = 128                 # s-chunk (stationary columns; full FWL width)
SCALE = 1.0 / math.sqrt(D)
BIGC = 300.0              # additive mask; exp(x - 300) == 0.0 in fp32
CLAMP = 30.0              # alpha clamp; valid alphas are O(5)

VDT = np.float16          # HBM dtype for VT/qT (test2 reads this)

_NC = {}
_SPEC = [None]            # tuple n_chunks[ML] of the last kernel() call


def _build(rep=1, spec=None):
    if spec is None:
        spec = _SPEC[0]
    assert spec is not None, "call kernel() first (spec comes from lengths)"
    nch = spec                     # per-slot s-chunk count (2..4), ascending
    key = (rep, nch)
    if key in _NC:
        return _NC[key]

    import concourse.bacc as bacc
    import concourse.tile as tile
    import concourse.mybir as mybir

    F32 = mybir.dt.float32
    F16 = mybir.dt.float16
    Op = mybir.AluOpType
    Act = mybir.ActivationFunctionType

    nc = bacc.Bacc(
        "TRN2", target_bir_lowering=False, debug=False, num_devices=NCORES
    )
    vt_ap = nc.dram_tensor("vt", [ML // 4, D, 4, S], F16, kind="ExternalInput").ap()
    qt_ap = nc.dram_tensor("qt", [D, ML], F16, kind="ExternalInput").ap()
    mk_ap = nc.dram_tensor("maskt", [S, ML], F32, kind="ExternalInput").ap()
    out_ap = nc.dram_tensor("out", [S, ML], F32, kind="ExternalOutput").ap()

    NSCH = S // SCH                # 4 s-chunk tiles of alpha_T

    # per-group chunk-major PSUM layout: for chunk c the slots needing it
    # form a suffix [k_c, GRP) (slots are length-sorted ascending)
    groups = []
    for g in range(NGRP):
        n = nch[g * GRP : (g + 1) * GRP]
        ks = [min((j for j in range(GRP) if n[j] > c), default=GRP)
              for c in range(NSCH)]
        offs = np.cumsum([0] + [GRP - k for k in ks]).tolist()
        groups.append((n, ks, offs))

    with tile.TileContext(nc) as tc:
        with (
            tc.tile_pool(name="pv", bufs=6) as pv,
            tc.tile_pool(name="pq", bufs=2) as pq,
            tc.tile_pool(name="pa", bufs=2) as pa,
            tc.tile_pool(name="ps", bufs=2) as ps,
            tc.tile_pool(name="pc", bufs=1) as pc,
            tc.tile_pool(name="pp", bufs=4, space="PSUM") as pp,
            tc.tile_pool(name="pp2", bufs=2, space="PSUM") as pp2,
        ):
            ones_col = pc.tile([128, 1], F32)       # lhsT for column sums
            nc.gpsimd.memset(ones_col[:], 1.0)
            ones_row = pc.tile([1, 128], F32)       # lhsT for 1/sum bcast
            nc.gpsimd.memset(ones_row[:], 1.0)

            for r in range(rep):
                qt_t = [pq.tile([128, ML], F16, tag=f"qt{dc}", name=f"qt{dc}") for dc in range(DCH)]
                for dc in range(DCH):
                    nc.scalar.dma_start(qt_t[dc][:], qt_ap[dc * 128 : (dc + 1) * 128, :])
                mk_t = [pq.tile([128, ML], F32, tag=f"mk{c}", name=f"mk{c}") for c in range(NSCH)]
                for c in range(NSCH):
                    nc.scalar.dma_start(mk_t[c][:], mk_ap[c * 128 : (c + 1) * 128, :])

                al_t = [pa.tile([128, ML], F32, tag=f"al{c}", name=f"al{c}") for c in range(NSCH)]
                # chunks 2/3 are not computed for short slots, and their
                # evacuation op (which adds the mask) never runs there —
                # preset to -300 so exp gives exactly 0 for those columns
                for c in range(2, NSCH):
                    nc.gpsimd.memset(al_t[c][:], -BIGC)

                for g in range(NGRP):
                    n, ks, offs = groups[g]
                    pg = pp.tile([128, 128], F32, tag="pg")
                    for j4 in range(GRP // 4):
                        g4 = g * (GRP // 4) + j4
                        # one 512KB DMA per (4-mention pack, d-chunk):
                        # 4KB contiguous per partition keeps the DMA on the
                        # large-line fast path (no s-trim on the transfer)
                        vt = pv.tile([128, DCH, 4, S], F16, tag="vt")
                        for dc in range(DCH):
                            nc.sync.dma_start(
                                vt[:, dc, :, :],
                                vt_ap[g4, dc * 128 : (dc + 1) * 128, :, :],
                            )
                        for i in range(4):
                            j = j4 * 4 + i
                            m = g * GRP + j
                            for c in range(n[j]):
                                col = offs[c] + (j - ks[c])
                                for dc in range(DCH):
                                    nc.tensor.matmul(
                                        pg[:, col : col + 1],
                                        vt[:, dc, i, c * SCH : (c + 1) * SCH],
                                        qt_t[dc][:, m : m + 1],
                                        start=(dc == 0),
                                        stop=(dc == DCH - 1),
                                    )
                    # evacuate: alpha_T = min(psum, 30) + mask  (one DVE op
                    # per chunk; scale by 1/sqrt(D) is folded into qT on host)
                    for c in range(NSCH):
                        cnt = GRP - ks[c]
                        if cnt == 0:
                            continue
                        nc.vector.scalar_tensor_tensor(
                            al_t[c][:, g * GRP + ks[c] : (g + 1) * GRP],
                            pg[:, offs[c] : offs[c] + cnt],
                            CLAMP,
                            mk_t[c][:, g * GRP + ks[c] : (g + 1) * GRP],
                            op0=Op.min,
                            op1=Op.add,
                        )

                # exp on ACT; column sums via PE ones-matmul
                ex_t = [pa.tile([128, ML], F32, tag=f"ex{c}", name=f"ex{c}") for c in range(NSCH)]
                sums = pp2.tile([1, ML], F32, tag="sums")
                for c in range(NSCH):
                    nc.scalar.activation(
                        ex_t[c][:], al_t[c][:], Act.Exp, bias=0.0, scale=1.0
                    )
                    nc.tensor.matmul(
                        sums[:],
                        ones_col[:],
                        ex_t[c][:],
                        start=(c == 0),
                        stop=(c == NSCH - 1),
                    )
                recip = ps.tile([1, ML], F32, tag="recip")
                nc.vector.reciprocal(recip[:], sums[:])
                bcast = pp2.tile([128, ML], F32, tag="bcast")
                nc.tensor.matmul(bcast[:], ones_row[:], recip[:])
                for c in range(NSCH):
                    ot = pa.tile([128, ML], F32, tag=f"ot{c}")
                    nc.vector.tensor_tensor(ot[:], ex_t[c][:], bcast[:], Op.mult)
                    nc.scalar.dma_start(out_ap[c * 128 : (c + 1) * 128, :], ot[:])

    nc.compile()
    _NC[key] = nc
    return nc


def timing_inputs(inputs):
    """Full (8-core concatenated) device arrays for the timing harness —
    content is irrelevant for the static schedule, shapes/dtypes matter."""
    return {
        "vt": np.zeros((NCORES * (ML // 4), D, 4, S), np.float16),
        "qt": np.zeros((NCORES * D, ML), np.float16),
        "maskt": np.zeros((NCORES * S, ML), np.float32),
    }


def _host_prep(idx, lengths):
    """Per-mention [len, start, end] int64."""
    idx = np.asarray(idx)
    lengths = np.asarray(lengths)
    sent = idx[:, 4].astype(np.int64)
    prefix = np.concatenate(
        [np.zeros(1, np.int64), np.cumsum(lengths.astype(np.int64))[:-1]]
    )
    mlen = lengths[sent].astype(np.int64)
    start = idx[:, 2].astype(np.int64) - prefix[sent]
    end = idx[:, 3].astype(np.int64) - prefix[sent]
    return mlen, start, end


def _plan(mlen):
    """Sort mentions by length, deal round-robin across cores; per-slot
    s-extent rounded to 128 (the PE stationary width)."""
    order = np.argsort(mlen, kind="stable")
    ls = mlen[order]
    slot_max = ls[7::8]                       # [ML] max len in each 8-rank slot
    nch = np.minimum((slot_max + SCH - 1) // SCH, S // SCH).astype(np.int64)
    return order, tuple(int(x) for x in nch)


def kernel(queries, values, idx, lengths):
    from concourse.bass_utils import run_bass_kernel_spmd

    queries = np.asarray(queries, dtype=np.float32)
    values = np.asarray(values, dtype=np.float32)
    mlen, mstart, mend = _host_prep(idx, lengths)
    order, spec = _plan(mlen)
    _SPEC[0] = spec

    # additive mask [M, S]: -300 where invalid
    pos = np.arange(S, dtype=np.int64)[None, :]
    invalid = (pos >= mlen[:, None]) | (
        (pos >= mstart[:, None]) & (pos < mend[:, None])
    )
    maskb = np.where(invalid, np.float32(-BIGC), np.float32(0.0))

    # fold the 1/sqrt(D) scale into q before the fp16 cast
    qs = (queries * np.float32(SCALE)).astype(np.float16)

    nc = _build(1, spec)
    in_maps = []
    sels = []
    for c in range(NCORES):
        sel = order[c::8]
        sels.append(sel)
        vt = np.ascontiguousarray(
            values[sel]
            .reshape(ML // 4, 4, S, D)
            .transpose(0, 3, 1, 2)
        ).astype(np.float16)                       # [ML/4, D, 4, S]
        qt = np.ascontiguousarray(qs[sel].T)       # [D, ML]
        mt = np.ascontiguousarray(maskb[sel].T)    # [S, ML]
        in_maps.append({"vt": vt, "qt": qt, "maskt": mt})
    res = run_bass_kernel_spmd(nc, in_maps, core_ids=list(range(NCORES)))
    out = np.empty((M, S), dtype=np.float32)
    for c in range(NCORES):
        out[sels[c]] = np.ascontiguousarray(res.results[c]["out"].T)
    return out
